# revision 22
# baseline (speedup 1.0000x reference)
"""Causal self-attention on Trainium2, tensor-parallel over heads across 8 NeuronCores.

Strategy (sharding_hint "tensor-parallel split the n_heads axis"):
  - Each core c owns heads {2c, 2c+1} == columns [128c, 128c+128) of Wq/Wk/Wv
    and rows [128c, 128c+128) of Wo.
  - Per core: QT/KT = (x @ W{q,k})^T in [feat, tok] layout (two heads stacked
    in one 128-partition tile), V computed in [feat, tok] layout with 512-col
    streams then PE-transposed into V_aug [tok, per-head (V(64)|ones(64))].
  - Scores are computed transposed ([k, q] layout, K=64 contraction) so
    exp(scoresT) feeds the PV matmul directly (lhsT = V_aug stationary,
    rhs = attnT streaming).  The 64 ones-columns of V_aug land the softmax
    denominator on PSUM partitions 64:128, so normalization is a DVE
    reciprocal + multiply with no cross-partition broadcast.
  - bk is dropped exactly (softmax is invariant to a per-query shift);
    bv is folded into the host-side bias (attn rows sum to 1, so
    out = PV_norm + bv and y += bv @ Wo); bq is applied for free as the
    per-partition bias of the scalar-engine Q evacuation.
  - Partial out-projection y_c = attnout_c @ Wo_c; host sums the 8 partials
    and adds bo + bv @ Wo.  (All-reduce done on host: gather/unshard step.)

Windows are processed in descending order per batch so the final window is
the smallest; filler work (next batch's QKV, previous tiles' out-proj) is
popped from an availability-driven queue between score/PV pair-groups to
keep the PE dense (HAM clock at 8/8) while the scalar engine's exp stream
hides underneath.

Matmul inputs are bf16 (PSUM accumulation fp32): single-pass matmuls + FWL
weight loads, vs fp32's LOW_HIGH double pass.
"""

import sys

if "/opt/trn_rl_repo" not in sys.path:
    sys.path.insert(0, "/opt/trn_rl_repo")

from contextlib import ExitStack

import ml_dtypes
import numpy as np

import concourse.bass as bass
import concourse.mybir as mybir
import concourse.tile as tile

F32 = mybir.dt.float32
BF = mybir.dt.bfloat16
NPBF = ml_dtypes.bfloat16
EXP = mybir.ActivationFunctionType.Exp
LN = mybir.ActivationFunctionType.Ln
IDENT = mybir.ActivationFunctionType.Identity

P = 128  # partition tile
HD = 64  # head dim
HC = 2  # heads per core (HC*HD == P)
WIN = 512  # token window (one PSUM bank of fp32)
N_WARM = 48  # PE warm-up matmuls (run under the x-load DMA shadow)
TARGET_GROUP = 7000  # PE cycles per attn pair-group incl. filler
TARGET_LAST = 11000  # last batch: drain out-proj aggressively (thin tail)


def _legalize_waits(nc):
    """This walrus build encodes at most ONE semaphore wait per instruction
    (setupSyncWait raises "Too many sync wait commands" otherwise).  Tile
    freely emits 2+ waits, so excess waits are moved onto injected same-engine
    NoOps (one wait each) directly before the instruction."""
    nop_id = 0
    for fn in nc.m.functions:
        for blk in fn.blocks:
            out = []
            for inst in blk.instructions:
                if type(inst).__name__ != "InstNoOp":
                    si = inst.sync_info
                    waits = list(si.on_wait or []) if si is not None else []
                    if len(waits) > 1:
                        for w in waits[1:]:
                            nop = mybir.InstNoOp(
                                name=f"nopw-{nop_id}",
                                engine=inst.engine,
                                ins=[],
                                outs=[],
                                sync_info=mybir.SyncInfo(on_wait=[w], on_update=[]),
                            )
                            nop_id += 1
                            out.append(nop)
                        si.on_wait = waits[:1]
                out.append(inst)
            blk.instructions[:] = out


def build_nc(B, T, D, n_cores, legalize=True):
    """Build the SPMD Bass program (same program all cores, per-core data)."""
    nj = D // P  # contraction tiles for projections
    n_win = T // WIN  # q windows per batch
    n_qt = T // P  # token tiles per batch
    tpw = n_qt // n_win  # token tiles per window
    M = B * T
    VW = 2 * P  # V_aug cols per token tile: per head [V(64) | ones(64)]

    nc = bass.Bass("TRN2", target_bir_lowering=False, debug=False, num_devices=n_cores)

    xt = nc.dram_tensor("xt", [D, M], BF, kind="ExternalInput").ap()
    wq = nc.dram_tensor("wq", [P, D], BF, kind="ExternalInput").ap()
    wk = nc.dram_tensor("wk", [P, D], BF, kind="ExternalInput").ap()
    wv = nc.dram_tensor("wv", [P, D], BF, kind="ExternalInput").ap()
    wo = nc.dram_tensor("wo", [P, D], BF, kind="ExternalInput").ap()
    bqc = nc.dram_tensor("bqc", [P, 1], F32, kind="ExternalInput").ap()
    msk = nc.dram_tensor("msk", [P, P], BF, kind="ExternalInput").ap()
    eye = nc.dram_tensor("eye", [P, P], BF, kind="ExternalInput").ap()
    y = nc.dram_tensor("y", [M, D], BF, kind="ExternalOutput").ap()

    with tile.TileContext(nc) as tc, ExitStack() as ctx:
        const = ctx.enter_context(tc.tile_pool(name="const", bufs=1))
        xtp = ctx.enter_context(tc.tile_pool(name="xt", bufs=2 * nj))
        qkp = ctx.enter_context(tc.tile_pool(name="qk", bufs=2))
        vtp = ctx.enter_context(tc.tile_pool(name="vt", bufs=2))
        vp = ctx.enter_context(tc.tile_pool(name="vaug", bufs=2))
        atp = ctx.enter_context(tc.tile_pool(name="attnT", bufs=8))
        aop = ctx.enter_context(tc.tile_pool(name="aoT", bufs=2))
        rcp = ctx.enter_context(tc.tile_pool(name="rc", bufs=4))
        yp = ctx.enter_context(tc.tile_pool(name="ysb", bufs=3))
        # PSUM budget (8 banks): sc 2x2-bank pairs + pv 2x1 + proj/y 2x1
        ps_sc = ctx.enter_context(tc.tile_pool(name="ps_sc", bufs=2, space="PSUM"))
        ps_pv = ctx.enter_context(tc.tile_pool(name="ps_pv", bufs=2, space="PSUM"))
        ps_proj = ctx.enter_context(tc.tile_pool(name="ps_proj", bufs=2, space="PSUM"))

        # constants / weights
        wq_s = const.tile([P, D], BF, tag="wq")
        wk_s = const.tile([P, D], BF, tag="wk")
        wv_s = const.tile([P, D], BF, tag="wv")
        wo_s = const.tile([P, D], BF, tag="wo")
        bq_s = const.tile([P, 1], F32, tag="bqc")
        msk_s = const.tile([P, P], BF, tag="msk")
        eye_s = const.tile([P, P], BF, tag="eye")
        warm_s = const.tile([P, WIN], BF, tag="warm")
        scr_s = const.tile([P, 1], BF, tag="scr")
        nc.vector.memset(warm_s[:, :], 1.0)
        # preload the Exp activation table off the critical path (into a
        # scratch tile so the warm-up matmuls don't wait on the table load)
        nc.scalar.activation(scr_s[:, :], warm_s[:, 0:1], EXP)
        nc.sync.dma_start(wq_s[:, :], wq[:, :])
        nc.sync.dma_start(wk_s[:, :], wk[:, :])
        nc.sync.dma_start(wv_s[:, :], wv[:, :])
        nc.sync.dma_start(wo_s[:, :], wo[:, :])
        nc.sync.dma_start(bq_s[:, :], bqc[:, :])
        nc.sync.dma_start(msk_s[:, :], msk[:, :])
        nc.sync.dma_start(eye_s[:, :], eye[:, :])

        # PE warm-up: dense dummy matmuls while the first x tiles stream in,
        # so the HAM clock gate reaches 8/8 before the real work starts.
        psw = ps_proj.tile([P, WIN], F32, tag="proj")
        for i in range(N_WARM):
            nc.tensor.matmul(
                psw[:, :], warm_s[:, 0:P], warm_s[:, :], start=True, stop=True
            )

        def scalar_recip(out, in_):
            # bass blocks ActivationFunctionType.Reciprocal for accuracy
            # reasons that don't apply here; emit the instruction directly.
            eng = nc.scalar
            ins = [eng.lower_ap(in_)]
            for arg in (0.0, 1.0, 0.0):  # bias, scale, alpha
                ins.append(mybir.ImmediateValue(dtype=mybir.dt.float32, value=arg))
            return eng.add_instruction(
                mybir.InstActivation(
                    name=nc.get_next_instruction_name(),
                    func=mybir.ActivationFunctionType.Reciprocal,
                    ins=ins,
                    outs=[eng.lower_ap(out)],
                )
            )

        st = {}  # per-batch pipeline state

        def load_xt(b):
            toff = b * T
            xts = []
            for j in range(nj):
                xt_t = xtp.tile([P, T], BF, tag="xt", name=f"xt{b}_{j}")
                nc.sync.dma_start(xt_t[:, :], xt[j * P : (j + 1) * P, toff : toff + T])
                xts.append(xt_t)
            vaug = vp.tile([P, n_qt * VW], BF, tag="vaug", name=f"vaug{b}")
            va3 = vaug.rearrange("p (t c) -> p t c", c=P)
            nc.vector.memset(va3[:, :, HD:P], 1.0)  # denominator ones-columns
            st[b] = {
                "xts": xts,
                "qt": qkp.tile([P, T], BF, tag="qt", name=f"qt{b}"),
                "kt": qkp.tile([P, T], BF, tag="kt", name=f"kt{b}"),
                "vaug": vaug,
                "aoT": aop.tile([P, T], BF, tag="aoT", name=f"aoT{b}"),
            }

        def proj_chunk(b, w, which):
            # qt/kt [feat, tok] with both heads stacked (rows 0:64 / 64:128)
            s = st[b]
            ws = w * WIN
            psp = ps_proj.tile([P, WIN], F32, tag="proj", name=f"ps{which}{b}_{w}")
            w_s = wq_s if which == "q" else wk_s
            for j in range(nj):
                nc.tensor.matmul(
                    psp[:, :],
                    w_s[:, j * P : (j + 1) * P],
                    s["xts"][j][:, ws : ws + WIN],
                    start=(j == 0),
                    stop=(j == nj - 1),
                )
            if which == "q":
                # evac on the DVE with the q bias as a per-partition scalar
                nc.vector.tensor_scalar_add(
                    s["qt"][:, ws : ws + WIN], psp[:, :], bq_s[:, 0:1]
                )
            else:
                nc.vector.tensor_copy(s["kt"][:, ws : ws + WIN], psp[:, :])

        def v_mm(b, w):
            # V projection in [feat, tok] layout: full 512-col streams
            s = st[b]
            ws = w * WIN
            psv = ps_proj.tile([P, WIN], F32, tag="proj", name=f"psv{b}_{w}")
            for j in range(nj):
                nc.tensor.matmul(
                    psv[:, :],
                    wv_s[:, j * P : (j + 1) * P],
                    s["xts"][j][:, ws : ws + WIN],
                    start=(j == 0),
                    stop=(j == nj - 1),
                )
            vt = vtp.tile([P, WIN], BF, tag="vt", name=f"vt{b}_{w}")
            nc.vector.tensor_copy(vt[:, :], psv[:, :])
            s[f"vt{w}"] = vt

        def v_tr(b, w):
            # PE-transpose vt [feat, tok] -> vaug [tok, (h0V|ones|h1V|ones)]
            s = st[b]
            vt = s.pop(f"vt{w}")
            pst = ps_proj.tile([P, WIN], BF, tag="proj", name=f"pst{b}_{w}")
            for q in range(tpw):
                nc.tensor.transpose(
                    pst[:, q * P : (q + 1) * P], vt[:, q * P : (q + 1) * P], eye_s[:, :]
                )
            # one strided copy: tile q, head h -> vaug col block
            src = pst.rearrange("p (q h f) -> p q h f", h=HC, f=HD)
            dst = s["vaug"].rearrange("p (q h f) -> p q h f", h=HC, f=VW // HC)
            nc.vector.tensor_copy(
                dst[:, w * tpw : (w + 1) * tpw, :, 0:HD], src[:, :, :, :]
            )

        def qkv_thunks(b):
            # v/k ascending (needed from k-tile 0 up), q descending (window
            # n-1 is processed first); v_tr two slots after its v_mm.
            qorder = list(reversed(range(n_win)))
            th = []
            for i, w in enumerate(range(n_win)):
                th.append((8 * WIN, lambda b=b, w=w: v_mm(b, w), "qkv"))
                th.append(
                    (8 * WIN, lambda b=b, w=qorder[i]: proj_chunk(b, w, "q"), "qkv")
                )
                th.append((tpw * P, lambda b=b, w=w: v_tr(b, w), "qkvt"))
                th.append((8 * WIN, lambda b=b, w=w: proj_chunk(b, w, "k"), "qkv"))
            return th

        # ---- filler queue: (pe_cycles, thunk, kind), popped between groups ----
        fillq = []

        def pop_filler(budget):
            got = 0
            while fillq and got < budget:
                cost, th, _ = fillq.pop(0)
                th()
                got += cost
            return got

        ydone = {}
        ydma_rr = [0]

        def y_dma_engine(n=2):
            ydma_rr[0] += 1
            return (nc.sync, nc.gpsimd, nc.scalar)[ydma_rr[0] % n]

        def outproj_tile(b, t):
            toff = b * T
            g = t // tpw  # output window group
            tl = t - g * tpw
            key = (b, g)
            if key not in ydone:
                ydone[key] = [
                    yp.tile([P, tpw * D], BF, tag="ysb", name=f"ysb{b}_{g}"), 0
                ]
            ysb, _ = ydone[key]
            aoT = st[b]["aoT"]
            for ui, u0 in enumerate(range(0, D, WIN)):
                psy = ps_proj.tile([P, WIN], F32, tag="proj", name=f"psy{b}_{t}_{ui}")
                nc.tensor.matmul(
                    psy[:, :],
                    aoT[:, t * P : (t + 1) * P],
                    wo_s[:, u0 : u0 + WIN],
                    start=True,
                    stop=True,
                )
                dst = ysb[:, tl * D + u0 : tl * D + u0 + WIN]
                if ui == 0 and b == B - 1 and g == 0:
                    # final window: the scalar engine is idle by now
                    nc.scalar.copy(dst, psy[:, :])
                else:
                    nc.vector.tensor_copy(dst, psy[:, :])
            last_win = b == B - 1 and g == 0
            if last_win:
                # final window: per-tile DMA for the shortest possible tail
                y_dma_engine(3).dma_start(
                    y[toff + t * P : toff + (t + 1) * P, :],
                    ysb[:, tl * D : (tl + 1) * D],
                )
                return
            ydone[key][1] += 1
            if ydone[key][1] == tpw:
                # one 1MB DMA per output window: y rows g*512 .. g*512+512
                yr = y.rearrange("(q p) d -> p q d", p=P)
                q0 = b * n_qt + g * tpw
                src = ysb.rearrange("p (q d) -> p q d", q=tpw)
                y_dma_engine().dma_start(yr[:, q0 : q0 + tpw, :], src)
                del ydone[key]

        def attn_window(b, w):
            target = TARGET_GROUP if b + 1 < B else TARGET_LAST
            # Heads interleaved; k tiles two at a time: both score chunks of a
            # head land in one 2-bank PSUM tile, one exp per pair; PV for pair
            # p is traced after the scores of pair p+1 so the PE never waits
            # on the exp.  Fillers popped after each PV flush.
            s = st[b]
            qt_s, kt_s, vaug = s["qt"], s["kt"], s["vaug"]
            ws = w * WIN
            njt = (ws + WIN) // P  # causal k tiles for this window
            pspv = [
                ps_pv.tile([P, WIN], F32, tag="pv", name=f"pspv{b}_{w}_{_h}")
                for _h in range(HC)
            ]

            def flush_pv(at, halves):
                for h in range(HC):
                    for j, off, N, qstart in halves[h]:
                        vb = j * VW + h * P
                        nc.tensor.matmul(
                            pspv[h][:, qstart - ws : WIN],
                            vaug[:, vb : vb + P],
                            at[h][:, off : off + N],
                            start=(j == 0),
                            stop=(j == njt - 1),
                        )

            prev = None
            for j0 in range(0, njt, 2):
                pss = [
                    ps_sc.tile([P, 2 * WIN], F32, tag="sc", name=f"pss{_h}")
                    for _h in range(HC)
                ]
                if not fillq:
                    # queue dry: dummy matmul into the score tile (about to
                    # be overwritten) keeps the HAM clock gate up.
                    nc.tensor.matmul(
                        pss[0][:, 0:WIN], warm_s[:, 0:P], warm_s[:, :],
                        start=True, stop=True,
                    )
                grp = 0
                at = [
                    atp.tile([P, 2 * WIN], BF, tag="at", name=f"at{_h}")
                    for _h in range(HC)
                ]
                halves = [[] for _ in range(HC)]
                off = [0] * HC
                for j in (j0, j0 + 1):
                    if j >= njt:
                        continue
                    qstart = max(ws, j * P)
                    N = ws + WIN - qstart
                    grp += 4 * N
                    for h in range(HC):
                        o = off[h]
                        if o and o + N > WIN:
                            o = WIN  # don't straddle a PSUM bank
                        nc.tensor.matmul(
                            pss[h][:, o : o + N],
                            kt_s[h * HD : (h + 1) * HD, j * P : (j + 1) * P],
                            qt_s[h * HD : (h + 1) * HD, qstart : qstart + N],
                            start=True,
                            stop=True,
                        )
                        halves[h].append((j, o, N, qstart))
                        off[h] = o + N
                for h in range(HC):
                    width = halves[h][-1][1] + halves[h][-1][2]
                    nc.scalar.activation(at[h][:, 0:width], pss[h][:, 0:width], EXP)
                    for j, o, N, qstart in halves[h]:
                        if j * P >= ws:  # zero the upper triangle post-exp
                            nc.vector.tensor_mul(
                                at[h][:, o : o + P], at[h][:, o : o + P],
                                msk_s[:, :],
                            )
                if prev is not None:
                    flush_pv(*prev)
                    pop_filler(max(0, target - grp))
                prev = (at, halves)
            flush_pv(*prev)
            # normalize: denominator rows (ones-columns of V_aug) sit on
            # PSUM partitions 64:128; scalar-engine reciprocal (runs right
            # after this window's last exp) + one DVE multiply from PSUM.
            for h in range(HC):
                rc = rcp.tile([HD, WIN], F32, tag="rc", name=f"rc{h}")
                scalar_recip(rc[:, :], pspv[h][HD:P, :])
                nc.vector.tensor_mul(
                    s["aoT"][h * HD : (h + 1) * HD, ws : ws + WIN],
                    pspv[h][0:HD, :],
                    rc[:, :],
                )
            pop_filler(target if b == B - 1 else target // 2)

        # ---- schedule: batch-0 QKV up front (under warm-up/DMA shadow),
        # then attention per batch in DESCENDING window order; QKV(b+1) and
        # out-proj thunks popped from the filler queue between pair-groups.
        load_xt(0)
        load_xt(1)
        for _, th, _k in qkv_thunks(0):
            th()
        for b in range(B):
            if b + 1 < B:
                fillq.extend(qkv_thunks(b + 1))
            for w in reversed(range(n_win)):
                if b > 0 and w == n_win - 1:
                    # QKV(b) must be done before attention on window n-1;
                    # drain v_tr thunks last so their PE transposes never
                    # wait on the just-traced DVE evacuations
                    if any(e[2].startswith("qkv") for e in fillq):
                        fillq.sort(key=lambda e: (e[2] == "qkvt", e[2] == "oproj"))
                        while any(e[2].startswith("qkv") for e in fillq):
                            pop_filler(1)
                attn_window(b, w)
                for t in range(w * tpw, (w + 1) * tpw):
                    fillq.append(
                        (2 * WIN, lambda b=b, t=t: outproj_tile(b, t), "oproj")
                    )
        while fillq:
            pop_filler(1)

    if legalize:
        _legalize_waits(nc)
    return nc


def make_in_maps(x, Wq, bq, Wk, bk, Wv, bv, Wo, n_cores):
    x = np.asarray(x, dtype=np.float32)
    Bb, Tt, Dd = x.shape
    M = Bb * Tt
    xt = np.ascontiguousarray(x.reshape(M, Dd).T.astype(NPBF))
    mask = np.where(
        np.arange(P)[:, None] > np.arange(P)[None, :], 0.0, 1.0
    ).astype(NPBF)
    ident = np.eye(P, dtype=NPBF)

    def wslice(W, c, scale=1.0):
        Wc = np.asarray(W, np.float32)[:, c * P : (c + 1) * P] * np.float32(scale)
        return np.ascontiguousarray(
            Wc.reshape(Dd // P, P, P).transpose(1, 0, 2).reshape(P, Dd).astype(NPBF)
        )

    qscale = 1.0 / np.sqrt(HD)
    in_maps = []
    for c in range(n_cores):
        cs = slice(c * P, (c + 1) * P)
        in_maps.append(
            {
                "xt": xt,
                "wq": wslice(Wq, c, qscale),
                "wk": wslice(Wk, c),
                "wv": wslice(Wv, c),
                "wo": np.ascontiguousarray(
                    np.asarray(Wo, np.float32)[cs, :].astype(NPBF)
                ),
                "bqc": np.ascontiguousarray(
                    (np.asarray(bq, np.float32)[cs] * np.float32(qscale))
                    .reshape(P, 1)
                ),
                "msk": mask,
                "eye": ident,
            }
        )
    return in_maps


_NC_CACHE = {}


def get_nc(B, T, D, n_cores):
    key = (B, T, D, n_cores)
    if key not in _NC_CACHE:
        _NC_CACHE[key] = build_nc(B, T, D, n_cores)
    return _NC_CACHE[key]


def kernel(**inputs):
    from concourse.bass_utils import run_bass_kernel_spmd

    x = np.asarray(inputs["x"], np.float32)
    Bb, Tt, Dd = x.shape
    n_cores = 8
    nc = get_nc(Bb, Tt, Dd, n_cores)
    in_maps = make_in_maps(
        x,
        inputs["Wq"],
        inputs["bq"],
        inputs["Wk"],
        inputs["bk"],
        inputs["Wv"],
        inputs["bv"],
        inputs["Wo"],
        n_cores,
    )
    res = run_bass_kernel_spmd(nc, in_maps, core_ids=list(range(n_cores)))
    y = np.zeros((Bb * Tt, Dd), dtype=np.float64)
    for r in res.results:
        y += r["y"].astype(np.float64)
    # bv is exact to fold here: attention rows sum to 1, so it contributes
    # bv @ Wo to every token; bk cancels in the softmax.
    y += (
        np.asarray(inputs["bo"], np.float64)
        + np.asarray(inputs["bv"], np.float64) @ np.asarray(inputs["Wo"], np.float64)
    )[None, :]
    return y.reshape(Bb, Tt, Dd).astype(np.float32)
